# revision 1
# baseline (speedup 1.0000x reference)
"""CAAN attention-scorer kernel for 8 Trainium2 NeuronCores.

scores = relu(softmax(QK^T/sqrt(D)) @ V @ W1 + b1) @ W2 + b2
with Q/K/V = x @ W{q,k,v} + b{q,k,v};  N=8192, IN_DIM=1024, D=512.

Row-parallel attention: core c owns rows [c*1024, (c+1)*1024). K/V are
computed (replicated) on every core from the full x^T — with fp8 DoubleRow
matmuls (2x PE rate) the replicated projections are cheaper than an
AllGather in this environment (~90us/call collective floor).

Everything runs in a transposed layout so softmax denominators come from a
constant-column PE matmul and normalization is deferred into the ctx PSUM
evacuation:

  kT[d, j]   = (8 Wk)^T x^T (+8 bk)   d on partitions   (fp8, x8 scale)
  qT[d, i]   = (8 Wq)^T xq^T (+8 bq)                    (fp8, x8 scale)
  v[j, d]    = x (8 Wv) (+8 bv)       j on partitions   (fp8 DRAM scratch)
  S^T[j, i]  = kT-chunk^T qT          (= 64 * true scores)
  E          = exp(S^T / (64 sqrt(D)))     PSUM -> SBUF fp8
  ctxU^T    += v-chunk^T E            (= 8 * true ctxU)
  den[1,i]  += (8.0)^T E              (= 8 * true denom -> cancels the x8)
  ctx^T      = ctxU^T * (1/den)       broadcast via ones (x) recip matmul
  h^T[m, i]  = relu(W1-chunk^T ctx^T + b1)    (bf16 MLP)
  out[1, i]  = W2^T h^T + b2

The x8 weight pre-scaling (exact power of two) keeps the uniform(+-1/32)
weights out of fp8-e4m3 denormal range. Measured end-to-end max-rel-err vs
the f32 reference: ~9e-3 (host simulation).
"""

import numpy as np
import ml_dtypes

import concourse.tile as tile
from concourse import bacc, mybir
from concourse.bass_utils import run_bass_kernel_spmd

N, IN_DIM, D = 8192, 1024, 512
NCORES = 8
NB = N // NCORES            # 1024 rows per core
P = 128
KC = IN_DIM // P            # 8  k-chunks of the input dim
KP = KC // 2                # 4  DoubleRow k-pairs
DC = D // P                 # 4  d-chunks
DP = DC // 2                # 2  DoubleRow d-pairs
MC = (D // 2) // P          # 2  m-chunks of the hidden dim
JT = 512                    # j-tile width in phase 1
NJT = N // JT               # 16
NJC = N // P                # 64 j-chunks
NJP = NJC // 2              # 32 j-pairs in phase 2
IH = 512                    # i-half width in phase 2
NIH = NB // IH              # 2
WS = 8.0                    # fp8 weight pre-scale (exact power of two)
SCALE = 1.0 / float(np.sqrt(np.float32(D))) / (WS * WS)

FP8 = mybir.dt.float8e4
BF16 = mybir.dt.bfloat16
F32 = mybir.dt.float32
DR = mybir.MatmulPerfMode.DoubleRow

_CACHE = {}


def _build():
    nc = bacc.Bacc(None, target_bir_lowering=False, debug=False)

    xt = nc.declare_dram_parameter("xt", [P, KC, N], FP8, isOutput=False)
    xq = nc.declare_dram_parameter("xq", [P, KC, NB], FP8, isOutput=False)
    wq = nc.declare_dram_parameter("wq", [P, KC, D], FP8, isOutput=False)
    wk = nc.declare_dram_parameter("wk", [P, KC, D], FP8, isOutput=False)
    wv = nc.declare_dram_parameter("wv", [P, KC, D], FP8, isOutput=False)
    w1 = nc.declare_dram_parameter("w1", [P, DC, MC, P], BF16, isOutput=False)
    w2 = nc.declare_dram_parameter("w2", [P, MC], BF16, isOutput=False)
    bq = nc.declare_dram_parameter("bq", [P, DC], F32, isOutput=False)   # x8
    bk = nc.declare_dram_parameter("bk", [P, DC], F32, isOutput=False)   # x8
    bv = nc.declare_dram_parameter("bv", [P, D], F32, isOutput=False)    # x8
    b1 = nc.declare_dram_parameter("b1", [P, MC], F32, isOutput=False)
    b2 = nc.declare_dram_parameter("b2", [1, 1], F32, isOutput=False)
    out = nc.declare_dram_parameter("out", [1, NB], F32, isOutput=True)

    with tile.TileContext(nc) as tc:
        with (
            tc.tile_pool(name="singles", bufs=1) as singles,
            tc.tile_pool(name="dram", bufs=1, space="DRAM") as dram,
        ):
            # ---- constants / weights into SBUF ----
            wq_sb = singles.tile([P, KC, D], FP8)
            wk_sb = singles.tile([P, KC, D], FP8)
            wv_sb = singles.tile([P, KC, D], FP8)
            w1_sb = singles.tile([P, DC, MC, P], BF16)
            w2_sb = singles.tile([P, MC], BF16)
            bq_sb = singles.tile([P, DC], F32)
            bk_sb = singles.tile([P, DC], F32)
            bv_sb = singles.tile([P, D], F32)
            b1_sb = singles.tile([P, MC], F32)
            b2_sb = singles.tile([1, 1], F32)
            cs_w = singles.tile([P, 2, 32], FP8)     # colsum weights = 8.0
            # (dual-fp8 ldweights needs >=32 active columns; rows identical)
            ones_f32 = singles.tile([1, P], F32)
            for kp in range(KP):
                nc.sync.dma_start(wq_sb[:, 2 * kp:2 * kp + 2],
                                  wq[:, 2 * kp:2 * kp + 2])
            nc.sync.dma_start(bq_sb[:], bq[:])
            for dst, src in [(wk_sb, wk), (bk_sb, bk), (wv_sb, wv), (bv_sb, bv),
                             (w1_sb, w1), (w2_sb, w2), (b1_sb, b1), (b2_sb, b2)]:
                nc.gpsimd.dma_start(out=dst[:], in_=src[:])
            nc.vector.memset(cs_w[:], WS)
            nc.vector.memset(ones_f32[:], 1.0)

            # persistent activations
            kt_sb = singles.tile([P, DC, N], FP8)       # kT, d on partitions
            qt_sb = singles.tile([P, DC, NB], FP8)      # qT
            v_sb = singles.tile([P, NJC, D], FP8)       # v, j on partitions

            # ---- all pools at top level: one fungible [128,512] PSUM tag so
            # phase-1 projection and phase-2 attention matmuls can interleave
            with (
                tc.tile_pool(name="xtiles", bufs=3) as xtiles,
                tc.tile_pool(name="evac", bufs=6) as evac,
                tc.tile_pool(name="etile", bufs=6) as etile,
                tc.tile_pool(name="mlp", bufs=2) as mlp,
                tc.tile_pool(name="ps_mm", bufs=3, space="PSUM") as ps_mm,
                tc.tile_pool(name="ps_ctx", bufs=1, space="PSUM") as ps_ctx,
                tc.tile_pool(name="ps_cs", bufs=1, space="PSUM") as ps_cs,
            ):
                # qT first: phase 2's S_T depends on it, so emitting it early
                # lets attention matmuls start as soon as kT tiles land
                for it in range(NB // JT):
                    xq_t = xtiles.tile([P, KC, JT], FP8, tag="xt")
                    if it == 0:
                        for kp in range(KP):
                            nc.sync.dma_start(
                                xq_t[:, 2 * kp:2 * kp + 2],
                                xq[:, 2 * kp:2 * kp + 2, it * JT:(it + 1) * JT])
                    else:
                        nc.sync.dma_start(xq_t[:],
                                          xq[:, :, it * JT:(it + 1) * JT])
                    for dc in range(DC):
                        ps = ps_mm.tile([P, JT], F32, tag="st")
                        for kp in range(KP):
                            nc.tensor.matmul(
                                ps[:],
                                wq_sb[:, 2 * kp:2 * kp + 2, dc * P:(dc + 1) * P],
                                xq_t[:, 2 * kp:2 * kp + 2],
                                start=(kp == 0), stop=(kp == KP - 1),
                                perf_mode=DR)
                        nc.vector.tensor_scalar_add(
                            qt_sb[:, dc, it * JT:(it + 1) * JT], ps[:],
                            bq_sb[:, dc:dc + 1])

                for jt in range(NJT):
                    xt_t = xtiles.tile([P, KC, JT], FP8, tag="xt")
                    if jt == 0:
                        for kp in range(KP):
                            nc.sync.dma_start(
                                xt_t[:, 2 * kp:2 * kp + 2],
                                xt[:, 2 * kp:2 * kp + 2, jt * JT:(jt + 1) * JT])
                    else:
                        nc.sync.dma_start(xt_t[:],
                                          xt[:, :, jt * JT:(jt + 1) * JT])
                    # kT tile: [P(d), JT] per d-chunk
                    for dc in range(DC):
                        ps = ps_mm.tile([P, JT], F32, tag="st")
                        for kp in range(KP):
                            nc.tensor.matmul(
                                ps[:],
                                wk_sb[:, 2 * kp:2 * kp + 2, dc * P:(dc + 1) * P],
                                xt_t[:, 2 * kp:2 * kp + 2],
                                start=(kp == 0), stop=(kp == KP - 1),
                                perf_mode=DR)
                        nc.vector.tensor_scalar_add(
                            kt_sb[:, dc, jt * JT:(jt + 1) * JT], ps[:],
                            bk_sb[:, dc:dc + 1])
                    # v chunks: [P(j), D] -> DRAM scratch
                    for jc in range(JT // P):
                        ps = ps_mm.tile([P, D], F32, tag="st")
                        for kp in range(KP):
                            nc.tensor.matmul(
                                ps[:],
                                xt_t[:, 2 * kp:2 * kp + 2, jc * P:(jc + 1) * P],
                                wv_sb[:, 2 * kp:2 * kp + 2],
                                start=(kp == 0), stop=(kp == KP - 1),
                                perf_mode=DR)
                        nc.vector.tensor_tensor(
                            v_sb[:, jt * (JT // P) + jc, :], ps[:], bv_sb[:],
                            mybir.AluOpType.add)

            # ---- phase 2: attention + MLP per i-half ----
                out_sb = singles.tile([1, NB], F32)
                for ih in range(NIH):
                    i0 = ih * IH
                    ctx_ps = ps_ctx.tile([P, DC, IH], F32)
                    cs_ps = ps_cs.tile([32, IH], F32)
                    for t in range(NJP):
                        e_t = etile.tile([P, 2, IH], FP8, tag="et")
                        for s in range(2):
                            jc = 2 * t + s
                            st_ps = ps_mm.tile([P, IH], F32, tag="st")
                            for dp in range(DP):
                                nc.tensor.matmul(
                                    st_ps[:],
                                    kt_sb[:, 2 * dp:2 * dp + 2,
                                          jc * P:(jc + 1) * P],
                                    qt_sb[:, 2 * dp:2 * dp + 2, i0:i0 + IH],
                                    start=(dp == 0), stop=(dp == DP - 1),
                                    perf_mode=DR)
                            nc.scalar.activation(
                                e_t[:, s], st_ps[:],
                                mybir.ActivationFunctionType.Exp,
                                bias=0.0, scale=SCALE)
                        nc.tensor.matmul(cs_ps[:], cs_w[:], e_t[:],
                                         start=(t == 0), stop=(t == NJP - 1),
                                         perf_mode=DR)
                        for dc in range(DC):
                            nc.tensor.matmul(
                                ctx_ps[:, dc],
                                v_sb[:, 2 * t:2 * t + 2, dc * P:(dc + 1) * P],
                                e_t[:],
                                start=(t == 0), stop=(t == NJP - 1),
                                perf_mode=DR)

                    # softmax denominators -> broadcast reciprocal
                    recip_sb = mlp.tile([1, IH], F32, tag="recip")
                    nc.vector.reciprocal(recip_sb[:], cs_ps[0:1])
                    r_ps = ps_mm.tile([P, IH], F32, tag="st")
                    nc.tensor.matmul(r_ps[:], ones_f32[:], recip_sb[:],
                                     start=True, stop=True)
                    r_sb = mlp.tile([P, IH], F32, tag="rsb")
                    nc.vector.tensor_copy(r_sb[:], r_ps[:])

                    # normalized ctx^T (bf16) at PSUM evacuation
                    ctxn = mlp.tile([P, DC, IH], BF16, tag="ctxn")
                    for dc in range(DC):
                        nc.vector.tensor_tensor(ctxn[:, dc], ctx_ps[:, dc],
                                                r_sb[:], mybir.AluOpType.mult)

                    # h^T = relu(W1-chunk^T ctx^T + b1); out = W2^T h^T + b2
                    sc_ps = ps_mm.tile([1, IH], F32, tag="st")
                    h_sb = mlp.tile([P, MC, IH], BF16, tag="hsb")
                    for mc in range(MC):
                        g_ps = ps_mm.tile([P, IH], F32, tag="st")
                        for dc in range(DC):
                            nc.tensor.matmul(g_ps[:], w1_sb[:, dc, mc],
                                             ctxn[:, dc],
                                             start=(dc == 0), stop=(dc == DC - 1))
                        nc.scalar.activation(h_sb[:, mc], g_ps[:],
                                             mybir.ActivationFunctionType.Relu,
                                             bias=b1_sb[:, mc:mc + 1], scale=1.0)
                    for mc in range(MC):
                        nc.tensor.matmul(sc_ps[:], w2_sb[:, mc:mc + 1],
                                         h_sb[:, mc],
                                         start=(mc == 0), stop=(mc == MC - 1))
                    nc.scalar.add(out_sb[:, i0:i0 + IH], sc_ps[:], b2_sb[:])

            nc.sync.dma_start(out[:], out_sb[:])

    nc.finalize()
    return nc


def _prep(inputs):
    """Host-side layout prep shared by all cores + per-core xq blocks."""
    f32 = np.float32
    bf16 = ml_dtypes.bfloat16
    fp8 = ml_dtypes.float8_e4m3
    x = np.ascontiguousarray(inputs["x"], dtype=f32)
    xt = np.ascontiguousarray(x.T)                                   # [IN, N]
    xt_r = np.ascontiguousarray(
        xt.reshape(KC, P, N).transpose(1, 0, 2).astype(fp8))         # [P, KC, N]

    def w_r(w):  # [IN, D] -> [P, KC, D], x8 scale into fp8 range
        return np.ascontiguousarray(
            (np.asarray(w, f32) * WS).reshape(KC, P, D)
            .transpose(1, 0, 2).astype(fp8))

    w1_r = np.ascontiguousarray(
        np.asarray(inputs["W1"], f32).reshape(DC, P, MC, P)
        .transpose(1, 0, 2, 3).astype(bf16))                         # [P, DC, MC, P]
    w2_r = np.ascontiguousarray(
        np.asarray(inputs["W2"], f32).reshape(MC, P).T.astype(bf16))  # [P, MC]

    def b_col(b, nchunks, scale=1.0):  # [nchunks*P] -> [P, nchunks]
        return np.ascontiguousarray(
            (np.asarray(b, f32) * scale).reshape(nchunks, P).T)

    shared = {
        "xt": xt_r,
        "wq": w_r(inputs["Wq"]),
        "wk": w_r(inputs["Wk"]),
        "wv": w_r(inputs["Wv"]),
        "w1": w1_r,
        "w2": w2_r,
        "bq": b_col(inputs["bq"], DC, WS),
        "bk": b_col(inputs["bk"], DC, WS),
        "bv": np.ascontiguousarray(
            np.broadcast_to(np.asarray(inputs["bv"], f32) * WS, (P, D))),
        "b1": b_col(inputs["b1"], MC),
        "b2": np.asarray(inputs["b2"], f32).reshape(1, 1),
    }
    xqs = [np.ascontiguousarray(xt_r[:, :, c * NB:(c + 1) * NB])
           for c in range(NCORES)]
    return shared, xqs


def kernel(**inputs) -> np.ndarray:
    if "nc" not in _CACHE:
        _CACHE["nc"] = _build()
    nc = _CACHE["nc"]
    shared, xqs = _prep(inputs)
    in_maps = [dict(shared, xq=xqs[c]) for c in range(NCORES)]
    res = run_bass_kernel_spmd(nc, in_maps, core_ids=list(range(NCORES)))
    return np.concatenate([res.results[c]["out"][0] for c in range(NCORES)])



# revision 9
# speedup vs baseline: 1.5226x; 1.5226x over previous
"""CAAN attention-scorer kernel for 8 Trainium2 NeuronCores (v2).

scores = relu(softmax(QK^T/sqrt(D)) @ V @ W1 + b1) @ W2 + b2
with Q/K/V = x @ W{q,k,v} + b{q,k,v};  N=8192, IN_DIM=1024, D=512.

Math restructure vs v1 (all exact up to fp8 rounding):
  * Wv@W1 folded on host (attention rows sum to 1):
      V'' = x @ (Wv W1) + (bv W1 + b1)  [N, 256]
      h   = relu(attn @ V'')            -- the whole first MLP layer is gone
  * bk dropped: a per-row constant shift of the logits cancels in softmax.
  * softmax normalization deferred past relu (relu(a)/c == relu(a/c), c>0)
    to the final [1, N] score vector: scores = (W2/2)^T relu(ctxU'')*recip.

Sharding (EXCHANGE=True): row-parallel attention. Each core projects ONLY
its own 1024 rows of kT / V'' (1/8 of the work v1 replicated 8x) and the
shards are all-gathered SBUF-to-SBUF with 7 XOR-addressed
remote_dma_broadcast sends per core (slot s holds the block of core
own^s; attention is order-invariant over j so XOR order needs no
unpermute). Entry race (a peer's send landing before this core cleared
its semaphores) is closed by a ready-handshake gated on the first K
evacuation (~8us in, >> inter-core launch skew); data sends wait for
ready==16. Exchange semaphores are cleared at kernel end so repeated
executions see a clean state.

EXCHANGE=False fallback: every core computes the full kT / V'' from the
full x^T (v1-style replication) — no cross-core traffic.

Tile's scheduling simulator cannot model remotely-incremented semaphores,
so waits on them are emitted with threshold 0 and patched to the real
value after TileContext exits (before finalize).

Layouts (x8 / x16 are exact power-of-two fp8 pre-scales):
  kT[d, j]   = (8 Wk)^T x^T             d on partitions   (fp8)
  qT[d, i]   = (8 Wq)^T xq^T (+8 bq)                      (fp8)
  v''[j, d'] = x (16 WvW1) (+16 b1'')   j on partitions   (fp8)
  S^T[j, i]  = kT-chunk^T qT            (= 64 * true)
  E          = exp(S^T / (64 sqrt(D)))  PSUM -> SBUF fp8, [P, 1024] ops
  ctxU''    += v''-chunk^T E            (= 16 * true)
  den[1,i]  += (8.0)^T E                (= 8 * sum E)
  raw[1, i]  = (W2/2)^T relu(ctxU'')    (= 8 * true raw)
  out        = raw * (1/den) + b2
"""

import numpy as np
import ml_dtypes

import concourse.tile as tile
from concourse import bacc, mybir
from concourse.bass_utils import run_bass_kernel_spmd

N, IN_DIM, D = 8192, 1024, 512
DP2 = D // 2                # 256 folded v'' width
NCORES = 8
NB = N // NCORES            # 1024 rows per core
P = 128
KC = IN_DIM // P            # 8 k-chunks of the input dim
KP = KC // 2                # 4 DoubleRow k-pairs
DC = D // P                 # 4 d-chunks (q/k)
DPAIR = DC // 2             # 2 DoubleRow d-pairs
VC = DP2 // P               # 2 d'-chunks (v'')
JT = 512                    # j-tile width in phase A
BJC = NB // P               # 8 j-chunks per block
BJP = BJC // 2              # 4 j-pairs per block
IH = 512                    # i-half width in phase B
NIH = NB // IH              # 2
WS = 8.0                    # fp8 weight pre-scale for Wq/Wk
WSV = 16.0                  # fp8 pre-scale for folded WvW1
SCALE = 1.0 / float(np.sqrt(np.float32(D))) / (WS * WS)

FP8 = mybir.dt.float8e4
BF16 = mybir.dt.bfloat16
F32 = mybir.dt.float32
DR = mybir.MatmulPerfMode.DoubleRow
ACT = mybir.ActivationFunctionType

EXCHANGE = False

_CACHE = {}


def _build(exchange=EXCHANGE):
    nc = bacc.Bacc(None, target_bir_lowering=False, debug=False,
                   num_devices=NCORES)

    xq = nc.declare_dram_parameter("xq", [P, KC, NB], FP8, isOutput=False)
    if not exchange:
        xt = nc.declare_dram_parameter("xt", [P, KC, N], FP8, isOutput=False)
    wq = nc.declare_dram_parameter("wq", [P, KC, D], FP8, isOutput=False)
    wk = nc.declare_dram_parameter("wk", [P, KC, D], FP8, isOutput=False)
    wv1 = nc.declare_dram_parameter("wv1", [P, KC, DP2], FP8, isOutput=False)
    bq = nc.declare_dram_parameter("bq", [P, DC], F32, isOutput=False)   # x8
    bvp = nc.declare_dram_parameter("bvp", [P, 4, DP2], F32,
                                    isOutput=False)  # x16
    w2 = nc.declare_dram_parameter("w2", [P, VC], BF16, isOutput=False)  # x0.5
    b2 = nc.declare_dram_parameter("b2", [1, 1], F32, isOutput=False)
    out = nc.declare_dram_parameter("out", [1, NB], F32, isOutput=True)

    patches = []   # (BassInstruction, real_wait_value)
    my_sems = []
    with tile.TileContext(nc) as tc:
        with tc.tile_pool(name="singles", bufs=1) as singles:
            # ---- weights / constants into SBUF ----
            wk_sb = singles.tile([P, KC, D], FP8)
            wq_sb = singles.tile([P, KC, D], FP8)
            wv1_sb = singles.tile([P, KC, DP2], FP8)
            bq_sb = singles.tile([P, DC], F32)
            bvp_sb = singles.tile([P, 4, DP2], F32)
            w2_sb = singles.tile([P, VC], BF16)
            b2_sb = singles.tile([1, 1], F32)
            cs_w = singles.tile([P, 2, 32], FP8)   # colsum weights = 8.0
            xq_sb = singles.tile([P, KC, NB], FP8)
            if not exchange:
                xt_sb = singles.tile([P, KC, N], FP8)
            qt_sb = singles.tile([P, DC, NB], FP8)
            # gathered kT / v'': slot s = block of core (own XOR s)
            # (global block s when exchange=False)
            ktg = singles.tile([P, NCORES, DC, NB], FP8)
            vg = singles.tile([P, NCORES, BJC, DP2], FP8)
            out_sb = singles.tile([1, NB], F32)

            nc.gpsimd.dma_start(out=wk_sb[:], in_=wk[:])
            for kp in range(KP):
                nc.sync.dma_start(xq_sb[:, 2 * kp:2 * kp + 2],
                                  xq[:, 2 * kp:2 * kp + 2])
            nc.gpsimd.dma_start(out=wv1_sb[:], in_=wv1[:])
            nc.gpsimd.dma_start(out=bvp_sb[:], in_=bvp[:])
            nc.gpsimd.dma_start(out=wq_sb[:], in_=wq[:])
            nc.gpsimd.dma_start(out=bq_sb[:], in_=bq[:])
            nc.gpsimd.dma_start(out=w2_sb[:], in_=w2[:])
            nc.gpsimd.dma_start(out=b2_sb[:], in_=b2[:])
            nc.vector.memset(cs_w[:], WS)
            if not exchange:
                for t in range(N // JT):
                    nc.sync.dma_start(
                        xt_sb[:, :, t * JT:(t + 1) * JT],
                        xt[:, :, t * JT:(t + 1) * JT])

            if exchange:
                rsems = [nc.alloc_semaphore(f"xch_{m}")
                         for m in range(1, NCORES)]
                ready = nc.alloc_semaphore("xch_ready")
                lsem = nc.alloc_semaphore("xch_local")
                my_sems = rsems + [ready, lsem]
                margin_dram = nc.dram_tensor("margin_scratch", [1, 2], FP8)

            # j-source for the kT / v'' projections
            if exchange:
                kv_src, kv_blocks = xq_sb, 1
            else:
                kv_src, kv_blocks = xt_sb, NCORES

            with (
                tc.tile_pool(name="st2", bufs=2, space="PSUM") as st2_pool,
                tc.tile_pool(name="ctxp", bufs=1, space="PSUM") as ctx_pool,
                tc.tile_pool(name="csp", bufs=1, space="PSUM") as cs_pool,
                tc.tile_pool(name="scp", bufs=1, space="PSUM") as sc_pool,
                tc.tile_pool(name="etile", bufs=6) as etile,
                tc.tile_pool(name="mlp", bufs=2) as mlp,
            ):
                # ---- phase A: kT / v'' projections (own block, or all) ----
                # kT -> ktg (no bias; bk cancels in softmax)
                for b in range(kv_blocks):
                    for jh in range(NB // JT):
                        j0 = b * NB + jh * JT
                        for dcp in range(DPAIR):
                            ps = st2_pool.tile([P, 2, JT], F32, tag="st")
                            for half in range(2):
                                dc = 2 * dcp + half
                                for kp in range(KP):
                                    nc.tensor.matmul(
                                        ps[:, half],
                                        wk_sb[:, 2 * kp:2 * kp + 2,
                                              dc * P:(dc + 1) * P],
                                        kv_src[:, 2 * kp:2 * kp + 2,
                                               j0:j0 + JT],
                                        start=(kp == 0), stop=(kp == KP - 1),
                                        perf_mode=DR)
                            nc.scalar.activation(
                                ktg[:, b, 2 * dcp:2 * dcp + 2,
                                    jh * JT:(jh + 1) * JT],
                                ps[:], ACT.Copy, bias=0.0, scale=1.0)

                # v'' -> vg (4 j-chunks per PSUM tile)
                for b in range(kv_blocks):
                    for g in range(BJC // 4):
                        ps = st2_pool.tile([P, 4, DP2], F32, tag="st")
                        for q in range(4):
                            jc0 = b * BJC + 4 * g + q
                            for kp in range(KP):
                                nc.tensor.matmul(
                                    ps[:, q],
                                    kv_src[:, 2 * kp:2 * kp + 2,
                                           jc0 * P:(jc0 + 1) * P],
                                    wv1_sb[:, 2 * kp:2 * kp + 2],
                                    start=(kp == 0), stop=(kp == KP - 1),
                                    perf_mode=DR)
                        nc.vector.tensor_tensor(
                            vg[:, b, 4 * g:4 * g + 4], ps[:],
                            bvp_sb[:], mybir.AluOpType.add)

                if exchange:
                    # ---- ready handshake + 14 XOR-addressed sends ----
                    nc.gpsimd.dma_start(margin_dram[:], ktg[0:1, 0, 0, 0:2])
                    nc.gpsimd.remote_sem_update_broadcast(
                        remote_sem=ready, local_sem=lsem,
                        rdests=[(0, k) for k in range(NCORES)])
                    nc.gpsimd.trigger_dma(count=1)
                    for m in range(1, NCORES):
                        rdests = [None] * 8
                        rdests[m] = (0, m)
                        nc.gpsimd.remote_dma_broadcast(
                            ktg[:, m], ktg[:, 0],
                            remote_sem=rsems[m - 1], local_sem=lsem,
                            rdests=rdests)
                        nc.gpsimd.remote_dma_broadcast(
                            vg[:, m], vg[:, 0],
                            remote_sem=rsems[m - 1], local_sem=lsem,
                            rdests=rdests)
                    w = nc.gpsimd.wait_ge(ready, 0)
                    patches.append((w, 2 * NCORES))
                    nc.gpsimd.trigger_dma(count=None)

                # qT (+8 bq) — after sends so kt/v'' shards go out early
                for it in range(NB // JT):
                    for dcp in range(DPAIR):
                        ps = st2_pool.tile([P, 2, JT], F32, tag="st")
                        for half in range(2):
                            dc = 2 * dcp + half
                            for kp in range(KP):
                                nc.tensor.matmul(
                                    ps[:, half],
                                    wq_sb[:, 2 * kp:2 * kp + 2,
                                          dc * P:(dc + 1) * P],
                                    xq_sb[:, 2 * kp:2 * kp + 2,
                                          it * JT:(it + 1) * JT],
                                    start=(kp == 0), stop=(kp == KP - 1),
                                    perf_mode=DR)
                        for half in range(2):
                            dc = 2 * dcp + half
                            nc.scalar.activation(
                                qt_sb[:, dc, it * JT:(it + 1) * JT],
                                ps[:, half], ACT.Identity,
                                bias=bq_sb[:, dc:dc + 1], scale=1.0)

                # ---- phase B: attention over the 8 blocks ----
                for ih in range(NIH):
                    i0 = ih * IH
                    ctx_ps = ctx_pool.tile([P, VC, IH], F32)
                    cs_ps = cs_pool.tile([32, IH], F32)
                    for s in range(NCORES):
                        if exchange and ih == 0 and s >= 1:
                            w = nc.tensor.wait_ge(rsems[s - 1], 0)
                            patches.append((w, 4))
                        for tp in range(BJP):
                            st = st2_pool.tile([P, 2, IH], F32, tag="st")
                            for half in range(2):
                                jc = 2 * tp + half
                                for dp in range(DPAIR):
                                    nc.tensor.matmul(
                                        st[:, half],
                                        ktg[:, s, 2 * dp:2 * dp + 2,
                                            jc * P:(jc + 1) * P],
                                        qt_sb[:, 2 * dp:2 * dp + 2,
                                              i0:i0 + IH],
                                        start=(dp == 0),
                                        stop=(dp == DPAIR - 1),
                                        perf_mode=DR)
                            e_t = etile.tile([P, 2, IH], FP8, tag="et")
                            nc.scalar.activation(e_t[:], st[:], ACT.Exp,
                                                 bias=0.0, scale=SCALE)
                            first = (s == 0 and tp == 0)
                            last = (s == NCORES - 1 and tp == BJP - 1)
                            nc.tensor.matmul(cs_ps[:], cs_w[:], e_t[:],
                                             start=first, stop=last,
                                             perf_mode=DR)
                            for vc in range(VC):
                                nc.tensor.matmul(
                                    ctx_ps[:, vc],
                                    vg[:, s, 2 * tp:2 * tp + 2,
                                       vc * P:(vc + 1) * P],
                                    e_t[:],
                                    start=first, stop=last,
                                    perf_mode=DR)

                    # tail: h = relu(ctxU''); raw = (W2/2)^T h; out = raw/den
                    h_sb = mlp.tile([P, VC, IH], BF16, tag="hsb")
                    nc.scalar.activation(h_sb[:], ctx_ps[:], ACT.Relu,
                                         bias=0.0, scale=1.0)
                    sc_ps = sc_pool.tile([1, IH], F32, tag="sc")
                    for vc in range(VC):
                        nc.tensor.matmul(sc_ps[:], w2_sb[:, vc:vc + 1],
                                         h_sb[:, vc],
                                         start=(vc == 0), stop=(vc == VC - 1))
                    recip = mlp.tile([1, IH], F32, tag="recip")
                    nc.vector.reciprocal(recip[:], cs_ps[0:1])
                    rawn = mlp.tile([1, IH], F32, tag="rawn")
                    nc.vector.tensor_tensor(rawn[:], sc_ps[:], recip[:],
                                            mybir.AluOpType.mult)
                    nc.scalar.add(out_sb[:, i0:i0 + IH], rawn[:], b2_sb[:])

            nc.sync.dma_start(out[:], out_sb[:])
            if exchange:
                # all sends flushed before teardown: 15 preps x 16
                w = nc.gpsimd.wait_ge(lsem, 0)
                patches.append((w, 15 * 16))

    if exchange:
        nc.has_collectives = True   # force NRT global comm init for RDMA
    for w, v in patches:
        w.ins.sync_info.on_wait[0].wait_value = v
    if my_sems:
        nc.clear_and_free_semaphores(my_sems)
    nc.finalize()
    return nc


def _prep(inputs):
    """Host-side layout prep: transposes, fp8 casts, Wv@W1 fold."""
    f32 = np.float32
    bf16 = ml_dtypes.bfloat16
    fp8 = ml_dtypes.float8_e4m3
    x = np.ascontiguousarray(inputs["x"], dtype=f32)
    xt_r = np.ascontiguousarray(
        x.T.reshape(KC, P, N).transpose(1, 0, 2).astype(fp8))    # [P, KC, N]

    def w_r(w, scale, cols):  # [IN, cols] -> [P, KC, cols], fp8 pre-scaled
        return np.ascontiguousarray(
            (np.asarray(w, f32) * scale).reshape(KC, P, cols)
            .transpose(1, 0, 2).astype(fp8))

    wv_w1 = np.asarray(inputs["Wv"], f32) @ np.asarray(inputs["W1"], f32)
    b1pp = (np.asarray(inputs["bv"], f32) @ np.asarray(inputs["W1"], f32)
            + np.asarray(inputs["b1"], f32))                     # [256]

    shared = {
        "wq": w_r(inputs["Wq"], WS, D),
        "wk": w_r(inputs["Wk"], WS, D),
        "wv1": w_r(wv_w1, WSV, DP2),
        "bq": np.ascontiguousarray(
            (np.asarray(inputs["bq"], f32) * WS).reshape(DC, P).T),
        "bvp": np.ascontiguousarray(
            np.broadcast_to(b1pp * WSV, (P, 4, DP2)).astype(f32)),
        "w2": np.ascontiguousarray(
            (np.asarray(inputs["W2"], f32) * 0.5)
            .reshape(VC, P).T.astype(bf16)),
        "b2": np.asarray(inputs["b2"], f32).reshape(1, 1),
    }
    if not EXCHANGE:
        shared["xt"] = xt_r
    xqs = [np.ascontiguousarray(xt_r[:, :, c * NB:(c + 1) * NB])
           for c in range(NCORES)]
    return shared, xqs


def kernel(**inputs) -> np.ndarray:
    if "nc" not in _CACHE:
        _CACHE["nc"] = _build()
    nc = _CACHE["nc"]
    shared, xqs = _prep(inputs)
    in_maps = [dict(shared, xq=xqs[c]) for c in range(NCORES)]
    res = run_bass_kernel_spmd(nc, in_maps, core_ids=list(range(NCORES)))
    return np.concatenate([res.results[c]["out"][0] for c in range(NCORES)])


# revision 13
# speedup vs baseline: 1.5855x; 1.0413x over previous
"""CAAN attention-scorer kernel for 8 Trainium2 NeuronCores (v2).

scores = relu(softmax(QK^T/sqrt(D)) @ V @ W1 + b1) @ W2 + b2
with Q/K/V = x @ W{q,k,v} + b{q,k,v};  N=8192, IN_DIM=1024, D=512.

Math restructure vs v1 (all exact up to fp8 rounding):
  * Wv@W1 folded on host (attention rows sum to 1):
      V'' = x @ (Wv W1) + (bv W1 + b1)  [N, 256]
      h   = relu(attn @ V'')            -- the whole first MLP layer is gone
  * bk dropped: a per-row constant shift of the logits cancels in softmax.
  * softmax normalization deferred past relu (relu(a)/c == relu(a/c), c>0)
    to the final [1, N] score vector: scores = (W2/2)^T relu(ctxU'')*recip.

Sharding (EXCHANGE=True): row-parallel attention. Each core projects ONLY
its own 1024 rows of kT / V'' (1/8 of the work v1 replicated 8x) and the
shards are all-gathered SBUF-to-SBUF with 7 XOR-addressed
remote_dma_broadcast sends per core (slot s holds the block of core
own^s; attention is order-invariant over j so XOR order needs no
unpermute). Entry race (a peer's send landing before this core cleared
its semaphores) is closed by a ready-handshake gated on the first K
evacuation (~8us in, >> inter-core launch skew); data sends wait for
ready==16. Exchange semaphores are cleared at kernel end so repeated
executions see a clean state.

EXCHANGE=False fallback: every core computes the full kT / V'' from the
full x^T (v1-style replication) — no cross-core traffic.

Tile's scheduling simulator cannot model remotely-incremented semaphores,
so waits on them are emitted with threshold 0 and patched to the real
value after TileContext exits (before finalize).

Layouts (x8 / x16 are exact power-of-two fp8 pre-scales):
  kT[d, j]   = (8 Wk)^T x^T             d on partitions   (fp8)
  qT[d, i]   = (8 Wq)^T xq^T (+8 bq)                      (fp8)
  v''[j, d'] = x (16 WvW1) (+16 b1'')   j on partitions   (fp8)
  S^T[j, i]  = kT-chunk^T qT            (= 64 * true)
  E          = exp(S^T / (64 sqrt(D)))  PSUM -> SBUF fp8, [P, 1024] ops
  ctxU''    += v''-chunk^T E            (= 16 * true)
  den[1,i]  += (8.0)^T E                (= 8 * sum E)
  raw[1, i]  = (W2/2)^T relu(ctxU'')    (= 8 * true raw)
  out        = raw * (1/den) + b2
"""

import numpy as np
import ml_dtypes

import concourse.tile as tile
from concourse import bacc, mybir
from concourse.bass_utils import run_bass_kernel_spmd

N, IN_DIM, D = 8192, 1024, 512
DP2 = D // 2                # 256 folded v'' width
NCORES = 8
NB = N // NCORES            # 1024 rows per core
P = 128
KC = IN_DIM // P            # 8 k-chunks of the input dim
KP = KC // 2                # 4 DoubleRow k-pairs
DC = D // P                 # 4 d-chunks (q/k)
DPAIR = DC // 2             # 2 DoubleRow d-pairs
VC = DP2 // P               # 2 d'-chunks (v'')
JT = 512                    # j-tile width in phase A
BJC = NB // P               # 8 j-chunks per block
BJP = BJC // 2              # 4 j-pairs per block
IH = 512                    # i-half width in phase B
NIH = NB // IH              # 2
WS = 8.0                    # fp8 weight pre-scale for Wq/Wk
WSV = 16.0                  # fp8 pre-scale for folded WvW1
SCALE = 1.0 / float(np.sqrt(np.float32(D))) / (WS * WS)

FP8 = mybir.dt.float8e4
BF16 = mybir.dt.bfloat16
F32 = mybir.dt.float32
DR = mybir.MatmulPerfMode.DoubleRow
ACT = mybir.ActivationFunctionType

EXCHANGE = False

_CACHE = {}


def _build(exchange=EXCHANGE):
    nc = bacc.Bacc(None, target_bir_lowering=False, debug=False,
                   num_devices=NCORES)

    xq = nc.declare_dram_parameter("xq", [P, KC, NB], FP8, isOutput=False)
    if not exchange:
        xt = nc.declare_dram_parameter("xt", [P, KC, N], FP8, isOutput=False)
    wq = nc.declare_dram_parameter("wq", [P, KC, D], FP8, isOutput=False)
    wk = nc.declare_dram_parameter("wk", [P, KC, D], FP8, isOutput=False)
    wv1 = nc.declare_dram_parameter("wv1", [P, KC, DP2], FP8, isOutput=False)
    bq = nc.declare_dram_parameter("bq", [P, DC], F32, isOutput=False)   # x8
    bvp = nc.declare_dram_parameter("bvp", [P, 4, DP2], F32,
                                    isOutput=False)  # x16
    w2 = nc.declare_dram_parameter("w2", [P, VC], BF16, isOutput=False)  # x0.5
    b2 = nc.declare_dram_parameter("b2", [1, 1], F32, isOutput=False)
    out = nc.declare_dram_parameter("out", [1, NB], F32, isOutput=True)

    patches = []   # (BassInstruction, real_wait_value)
    my_sems = []
    with tile.TileContext(nc) as tc:
        with tc.tile_pool(name="singles", bufs=1) as singles:
            # ---- weights / constants into SBUF ----
            wk_sb = singles.tile([P, KC, D], FP8)
            wq_sb = singles.tile([P, KC, D], FP8)
            wv1_sb = singles.tile([P, KC, DP2], FP8)
            bq_sb = singles.tile([P, DC], F32)
            bvp_sb = singles.tile([P, 4, DP2], F32)
            w2_sb = singles.tile([P, VC], BF16)
            b2_sb = singles.tile([1, 1], F32)
            cs_w = singles.tile([P, 2, 32], FP8)   # colsum weights = 8.0
            xq_sb = singles.tile([P, KC, NB], FP8)
            if not exchange:
                xt_sb = singles.tile([P, KC, N], FP8)
            qt_sb = singles.tile([P, DC, NB], FP8)
            # gathered kT / v'': slot s = block of core (own XOR s)
            # (global block s when exchange=False)
            ktg = singles.tile([P, NCORES, DC, NB], FP8)
            vg = singles.tile([P, NCORES, BJC, DP2], FP8)
            out_sb = singles.tile([1, NB], F32)

            # DMA order tuned so Q can start immediately and kt follows:
            # xq (sync) + wq/bq (gpsimd) land first, wk next, then xt.
            for kp in range(KP):
                nc.sync.dma_start(xq_sb[:, 2 * kp:2 * kp + 2],
                                  xq[:, 2 * kp:2 * kp + 2])
            nc.gpsimd.dma_start(out=wq_sb[:], in_=wq[:])
            nc.gpsimd.dma_start(out=bq_sb[:], in_=bq[:])
            nc.gpsimd.dma_start(out=wk_sb[:], in_=wk[:])
            nc.gpsimd.dma_start(out=wv1_sb[:], in_=wv1[:])
            nc.gpsimd.dma_start(out=bvp_sb[:], in_=bvp[:])
            nc.gpsimd.dma_start(out=w2_sb[:], in_=w2[:])
            nc.gpsimd.dma_start(out=b2_sb[:], in_=b2[:])
            nc.vector.memset(cs_w[:], WS)
            if not exchange:
                for t in range(N // JT):
                    nc.sync.dma_start(
                        xt_sb[:, :, t * JT:(t + 1) * JT],
                        xt[:, :, t * JT:(t + 1) * JT])

            if exchange:
                rsems = [nc.alloc_semaphore(f"xch_{m}")
                         for m in range(1, NCORES)]
                ready = nc.alloc_semaphore("xch_ready")
                lsem = nc.alloc_semaphore("xch_local")
                my_sems = rsems + [ready, lsem]
                margin_dram = nc.dram_tensor("margin_scratch", [1, 2], FP8)

            # j-source for the kT / v'' projections
            if exchange:
                kv_src, kv_blocks = xq_sb, 1
            else:
                kv_src, kv_blocks = xt_sb, NCORES

            with (
                tc.tile_pool(name="st2", bufs=2, space="PSUM") as st2_pool,
                tc.tile_pool(name="ctxp", bufs=1, space="PSUM") as ctx_pool,
                tc.tile_pool(name="csp", bufs=1, space="PSUM") as cs_pool,
                tc.tile_pool(name="scp", bufs=1, space="PSUM") as sc_pool,
                tc.tile_pool(name="etile", bufs=6) as etile,
                tc.tile_pool(name="mlp", bufs=2) as mlp,
            ):
                # ---- phase A ----
                # qT (+8 bq) FIRST: only needs xq/wq, runs while xt lands
                for it in range(NB // JT):
                    for dcp in range(DPAIR):
                        ps = st2_pool.tile([P, 2, JT], F32, tag="st")
                        for half in range(2):
                            dc = 2 * dcp + half
                            for kp in range(KP):
                                nc.tensor.matmul(
                                    ps[:, half],
                                    wq_sb[:, 2 * kp:2 * kp + 2,
                                          dc * P:(dc + 1) * P],
                                    xq_sb[:, 2 * kp:2 * kp + 2,
                                          it * JT:(it + 1) * JT],
                                    start=(kp == 0), stop=(kp == KP - 1),
                                    perf_mode=DR)
                        for half in range(2):
                            dc = 2 * dcp + half
                            nc.scalar.activation(
                                qt_sb[:, dc, it * JT:(it + 1) * JT],
                                ps[:, half], ACT.Identity,
                                bias=bq_sb[:, dc:dc + 1], scale=1.0)

                # kT / v'' per 512-column tile, paired with the xt DMA order
                for b in range(kv_blocks):
                    for jh in range(NB // JT):
                        j0 = b * NB + jh * JT
                        # kT (no bias; bk cancels in softmax)
                        for dcp in range(DPAIR):
                            ps = st2_pool.tile([P, 2, JT], F32, tag="st")
                            for half in range(2):
                                dc = 2 * dcp + half
                                for kp in range(KP):
                                    nc.tensor.matmul(
                                        ps[:, half],
                                        wk_sb[:, 2 * kp:2 * kp + 2,
                                              dc * P:(dc + 1) * P],
                                        kv_src[:, 2 * kp:2 * kp + 2,
                                               j0:j0 + JT],
                                        start=(kp == 0), stop=(kp == KP - 1),
                                        perf_mode=DR)
                            nc.scalar.activation(
                                ktg[:, b, 2 * dcp:2 * dcp + 2,
                                    jh * JT:(jh + 1) * JT],
                                ps[:], ACT.Copy, bias=0.0, scale=1.0)
                        # v'' for the same 4 j-chunks
                        ps = st2_pool.tile([P, 4, DP2], F32, tag="st")
                        for q in range(4):
                            jc0 = b * BJC + 4 * jh + q
                            for kp in range(KP):
                                nc.tensor.matmul(
                                    ps[:, q],
                                    kv_src[:, 2 * kp:2 * kp + 2,
                                           jc0 * P:(jc0 + 1) * P],
                                    wv1_sb[:, 2 * kp:2 * kp + 2],
                                    start=(kp == 0), stop=(kp == KP - 1),
                                    perf_mode=DR)
                        nc.vector.tensor_tensor(
                            vg[:, b, 4 * jh:4 * jh + 4], ps[:],
                            bvp_sb[:], mybir.AluOpType.add)

                if exchange:
                    # ---- ready handshake + 14 XOR-addressed sends ----
                    nc.gpsimd.dma_start(margin_dram[:], ktg[0:1, 0, 0, 0:2])
                    nc.gpsimd.remote_sem_update_broadcast(
                        remote_sem=ready, local_sem=lsem,
                        rdests=[(0, k) for k in range(NCORES)])
                    nc.gpsimd.trigger_dma(count=1)
                    for m in range(1, NCORES):
                        rdests = [None] * 8
                        rdests[m] = (0, m)
                        nc.gpsimd.remote_dma_broadcast(
                            ktg[:, m], ktg[:, 0],
                            remote_sem=rsems[m - 1], local_sem=lsem,
                            rdests=rdests)
                        nc.gpsimd.remote_dma_broadcast(
                            vg[:, m], vg[:, 0],
                            remote_sem=rsems[m - 1], local_sem=lsem,
                            rdests=rdests)
                    w = nc.gpsimd.wait_ge(ready, 0)
                    patches.append((w, 2 * NCORES))
                    nc.gpsimd.trigger_dma(count=None)

                # ---- phase B: attention over the 8 blocks ----
                for ih in range(NIH):
                    i0 = ih * IH
                    ctx_ps = ctx_pool.tile([P, VC, IH], F32)
                    cs_ps = cs_pool.tile([32, IH], F32)
                    for s in range(NCORES):
                        if exchange and ih == 0 and s >= 1:
                            w = nc.tensor.wait_ge(rsems[s - 1], 0)
                            patches.append((w, 4))
                        for tp in range(BJP):
                            st = st2_pool.tile([P, 2, IH], F32, tag="st")
                            for half in range(2):
                                jc = 2 * tp + half
                                for dp in range(DPAIR):
                                    nc.tensor.matmul(
                                        st[:, half],
                                        ktg[:, s, 2 * dp:2 * dp + 2,
                                            jc * P:(jc + 1) * P],
                                        qt_sb[:, 2 * dp:2 * dp + 2,
                                              i0:i0 + IH],
                                        start=(dp == 0),
                                        stop=(dp == DPAIR - 1),
                                        perf_mode=DR)
                            e_t = etile.tile([P, 2, IH], FP8, tag="et")
                            nc.scalar.activation(e_t[:], st[:], ACT.Exp,
                                                 bias=0.0, scale=SCALE)
                            first = (s == 0 and tp == 0)
                            last = (s == NCORES - 1 and tp == BJP - 1)
                            nc.tensor.matmul(cs_ps[:], cs_w[:], e_t[:],
                                             start=first, stop=last,
                                             perf_mode=DR)
                            for vc in range(VC):
                                nc.tensor.matmul(
                                    ctx_ps[:, vc],
                                    vg[:, s, 2 * tp:2 * tp + 2,
                                       vc * P:(vc + 1) * P],
                                    e_t[:],
                                    start=first, stop=last,
                                    perf_mode=DR)

                    # tail: h = relu(ctxU''); raw = (W2/2)^T h; out = raw/den
                    h_sb = mlp.tile([P, VC, IH], BF16, tag="hsb")
                    nc.scalar.activation(h_sb[:], ctx_ps[:], ACT.Relu,
                                         bias=0.0, scale=1.0)
                    sc_ps = sc_pool.tile([1, IH], F32, tag="sc")
                    for vc in range(VC):
                        nc.tensor.matmul(sc_ps[:], w2_sb[:, vc:vc + 1],
                                         h_sb[:, vc],
                                         start=(vc == 0), stop=(vc == VC - 1))
                    recip = mlp.tile([1, IH], F32, tag="recip")
                    nc.vector.reciprocal_approx_fast(recip[:], cs_ps[0:1])
                    rawn = mlp.tile([1, IH], F32, tag="rawn")
                    nc.vector.tensor_tensor(rawn[:], sc_ps[:], recip[:],
                                            mybir.AluOpType.mult)
                    nc.scalar.add(out_sb[:, i0:i0 + IH], rawn[:], b2_sb[:])

            nc.sync.dma_start(out[:], out_sb[:])
            if exchange:
                # all sends flushed before teardown: 15 preps x 16
                w = nc.gpsimd.wait_ge(lsem, 0)
                patches.append((w, 15 * 16))

    if exchange:
        nc.has_collectives = True   # force NRT global comm init for RDMA
    for w, v in patches:
        w.ins.sync_info.on_wait[0].wait_value = v
    if my_sems:
        nc.clear_and_free_semaphores(my_sems)
    nc.finalize()
    return nc


def _prep(inputs):
    """Host-side layout prep: transposes, fp8 casts, Wv@W1 fold."""
    f32 = np.float32
    bf16 = ml_dtypes.bfloat16
    fp8 = ml_dtypes.float8_e4m3
    x = np.ascontiguousarray(inputs["x"], dtype=f32)
    xt_r = np.ascontiguousarray(
        x.T.reshape(KC, P, N).transpose(1, 0, 2).astype(fp8))    # [P, KC, N]

    def w_r(w, scale, cols):  # [IN, cols] -> [P, KC, cols], fp8 pre-scaled
        return np.ascontiguousarray(
            (np.asarray(w, f32) * scale).reshape(KC, P, cols)
            .transpose(1, 0, 2).astype(fp8))

    wv_w1 = np.asarray(inputs["Wv"], f32) @ np.asarray(inputs["W1"], f32)
    b1pp = (np.asarray(inputs["bv"], f32) @ np.asarray(inputs["W1"], f32)
            + np.asarray(inputs["b1"], f32))                     # [256]

    shared = {
        "wq": w_r(inputs["Wq"], WS, D),
        "wk": w_r(inputs["Wk"], WS, D),
        "wv1": w_r(wv_w1, WSV, DP2),
        "bq": np.ascontiguousarray(
            (np.asarray(inputs["bq"], f32) * WS).reshape(DC, P).T),
        "bvp": np.ascontiguousarray(
            np.broadcast_to(b1pp * WSV, (P, 4, DP2)).astype(f32)),
        "w2": np.ascontiguousarray(
            (np.asarray(inputs["W2"], f32) * 0.5)
            .reshape(VC, P).T.astype(bf16)),
        "b2": np.asarray(inputs["b2"], f32).reshape(1, 1),
    }
    if not EXCHANGE:
        shared["xt"] = xt_r
    xqs = [np.ascontiguousarray(xt_r[:, :, c * NB:(c + 1) * NB])
           for c in range(NCORES)]
    return shared, xqs


def kernel(**inputs) -> np.ndarray:
    if "nc" not in _CACHE:
        _CACHE["nc"] = _build()
    nc = _CACHE["nc"]
    shared, xqs = _prep(inputs)
    in_maps = [dict(shared, xq=xqs[c]) for c in range(NCORES)]
    res = run_bass_kernel_spmd(nc, in_maps, core_ids=list(range(NCORES)))
    return np.concatenate([res.results[c]["out"][0] for c in range(NCORES)])


# revision 20
# speedup vs baseline: 1.5896x; 1.0026x over previous
"""CAAN attention-scorer kernel for 8 Trainium2 NeuronCores (v2).

scores = relu(softmax(QK^T/sqrt(D)) @ V @ W1 + b1) @ W2 + b2
with Q/K/V = x @ W{q,k,v} + b{q,k,v};  N=8192, IN_DIM=1024, D=512.

Math restructure vs v1 (all exact up to fp8 rounding):
  * Wv@W1 folded on host (attention rows sum to 1):
      V'' = x @ (Wv W1) + (bv W1 + b1)  [N, 256]
      h   = relu(attn @ V'')            -- the whole first MLP layer is gone
  * bk dropped: a per-row constant shift of the logits cancels in softmax.
  * softmax normalization deferred past relu (relu(a)/c == relu(a/c), c>0)
    to the final [1, N] score vector: scores = (W2/2)^T relu(ctxU'')*recip.

Sharding (EXCHANGE=True): row-parallel attention. Each core projects ONLY
its own 1024 rows of kT / V'' (1/8 of the work v1 replicated 8x) and the
shards are all-gathered SBUF-to-SBUF with 7 XOR-addressed
remote_dma_broadcast sends per core (slot s holds the block of core
own^s; attention is order-invariant over j so XOR order needs no
unpermute). Entry race (a peer's send landing before this core cleared
its semaphores) is closed by a ready-handshake gated on the first K
evacuation (~8us in, >> inter-core launch skew); data sends wait for
ready==16. Exchange semaphores are cleared at kernel end so repeated
executions see a clean state.

EXCHANGE=False fallback: every core computes the full kT / V'' from the
full x^T (v1-style replication) — no cross-core traffic.

Tile's scheduling simulator cannot model remotely-incremented semaphores,
so waits on them are emitted with threshold 0 and patched to the real
value after TileContext exits (before finalize).

Layouts (x8 / x16 are exact power-of-two fp8 pre-scales):
  kT[d, j]   = (8 Wk)^T x^T             d on partitions   (fp8)
  qT[d, i]   = (8 Wq)^T xq^T (+8 bq)                      (fp8)
  v''[j, d'] = x (16 WvW1) (+16 b1'')   j on partitions   (fp8)
  S^T[j, i]  = kT-chunk^T qT            (= 64 * true)
  E          = exp(S^T / (64 sqrt(D)))  PSUM -> SBUF fp8, [P, 1024] ops
  ctxU''    += v''-chunk^T E            (= 16 * true)
  den[1,i]  += (8.0)^T E                (= 8 * sum E)
  raw[1, i]  = (W2/2)^T relu(ctxU'')    (= 8 * true raw)
  out        = raw * (1/den) + b2
"""

import numpy as np
import ml_dtypes

import concourse.tile as tile
from concourse import bacc, mybir
from concourse.bass_utils import run_bass_kernel_spmd

N, IN_DIM, D = 8192, 1024, 512
DP2 = D // 2                # 256 folded v'' width
NCORES = 8
NB = N // NCORES            # 1024 rows per core
P = 128
KC = IN_DIM // P            # 8 k-chunks of the input dim
KP = KC // 2                # 4 DoubleRow k-pairs
DC = D // P                 # 4 d-chunks (q/k)
DPAIR = DC // 2             # 2 DoubleRow d-pairs
VC = DP2 // P               # 2 d'-chunks (v'')
JT = 512                    # j-tile width in phase A
BJC = NB // P               # 8 j-chunks per block
BJP = BJC // 2              # 4 j-pairs per block
IH = 512                    # i-half width in phase B
NIH = NB // IH              # 2
WS = 8.0                    # fp8 weight pre-scale for Wq/Wk
WSV = 16.0                  # fp8 pre-scale for folded WvW1
SCALE = 1.0 / float(np.sqrt(np.float32(D))) / (WS * WS)

FP8 = mybir.dt.float8e4
BF16 = mybir.dt.bfloat16
F32 = mybir.dt.float32
DR = mybir.MatmulPerfMode.DoubleRow
ACT = mybir.ActivationFunctionType

EXCHANGE = False

_CACHE = {}


def _build(exchange=EXCHANGE):
    nc = bacc.Bacc(None, target_bir_lowering=False, debug=False,
                   num_devices=NCORES)

    NT = N // JT
    xq = nc.declare_dram_parameter("xq", [P, KC, NB], FP8, isOutput=False)
    if not exchange:
        # chunk-major so each 512-column tile is one contiguous DMA
        xt = nc.declare_dram_parameter("xt", [NT, P, KC, JT], FP8,
                                       isOutput=False)
    wq = nc.declare_dram_parameter("wq", [P, KC, D], FP8, isOutput=False)
    wk = nc.declare_dram_parameter("wk", [P, KC, D], FP8, isOutput=False)
    wv1 = nc.declare_dram_parameter("wv1", [P, KC, DP2], FP8, isOutput=False)
    bq = nc.declare_dram_parameter("bq", [P, DC], F32, isOutput=False)   # x8
    bvp = nc.declare_dram_parameter("bvp", [P, 4, DP2], F32,
                                    isOutput=False)  # x16
    w2 = nc.declare_dram_parameter("w2", [P, VC], BF16, isOutput=False)  # x0.5
    b2 = nc.declare_dram_parameter("b2", [1, 1], F32, isOutput=False)
    out = nc.declare_dram_parameter("out", [1, NB], F32, isOutput=True)

    patches = []   # (BassInstruction, real_wait_value)
    my_sems = []
    with tile.TileContext(nc) as tc:
        with tc.tile_pool(name="singles", bufs=1) as singles:
            # ---- weights / constants into SBUF ----
            wk_sb = singles.tile([P, KC, D], FP8)
            wq_sb = singles.tile([P, KC, D], FP8)
            wv1_sb = singles.tile([P, KC, DP2], FP8)
            bq_sb = singles.tile([P, DC], F32)
            bvp_sb = singles.tile([P, 4, DP2], F32)
            w2_sb = singles.tile([P, VC], BF16)
            b2_sb = singles.tile([1, 1], F32)
            cs_w = singles.tile([P, 2, 32], FP8)   # colsum weights = 8.0
            xq_sb = singles.tile([P, KC, NB], FP8)
            if not exchange:
                xt_sb = singles.tile([P, NT, KC, JT], FP8)
            qt_sb = singles.tile([P, DC, NB], FP8)
            # gathered kT / v'': slot s = block of core (own XOR s)
            # (global block s when exchange=False)
            ktg = singles.tile([P, NCORES, DC, NB], FP8)
            vg = singles.tile([P, NCORES, BJC, DP2], FP8)
            out_sb = singles.tile([1, NB], F32)

            # DMA order tuned so Q can start immediately and kt follows:
            # xq (sync) + wq/bq (gpsimd) land first, wk next, then xt.
            for kp in range(KP):
                nc.sync.dma_start(xq_sb[:, 2 * kp:2 * kp + 2],
                                  xq[:, 2 * kp:2 * kp + 2])
            for kp in range(KP):
                nc.gpsimd.dma_start(out=wq_sb[:, 2 * kp:2 * kp + 2],
                                    in_=wq[:, 2 * kp:2 * kp + 2])
            nc.gpsimd.dma_start(out=bq_sb[:], in_=bq[:])
            for kp in range(KP):
                nc.gpsimd.dma_start(out=wk_sb[:, 2 * kp:2 * kp + 2],
                                    in_=wk[:, 2 * kp:2 * kp + 2])
            nc.gpsimd.dma_start(out=wv1_sb[:], in_=wv1[:])
            nc.gpsimd.dma_start(out=bvp_sb[:], in_=bvp[:])
            nc.gpsimd.dma_start(out=w2_sb[:], in_=w2[:])
            nc.gpsimd.dma_start(out=b2_sb[:], in_=b2[:])
            nc.vector.memset(cs_w[:], WS)
            if not exchange:
                for t in range(NT):
                    nc.sync.dma_start(xt_sb[:, t], xt[t])

            if exchange:
                rsems = [nc.alloc_semaphore(f"xch_{m}")
                         for m in range(1, NCORES)]
                ready = nc.alloc_semaphore("xch_ready")
                lsem = nc.alloc_semaphore("xch_local")
                my_sems = rsems + [ready, lsem]
                margin_dram = nc.dram_tensor("margin_scratch", [1, 2], FP8)

            # j-source slices for the kT / v'' projections
            if exchange:
                kv_blocks = 1

                def k_mov(t, kp):
                    return xq_sb[:, 2 * kp:2 * kp + 2,
                                 (t % 2) * JT:(t % 2 + 1) * JT]

                def v_mov(t, kp, q):
                    jc = (t % 2) * 4 + q
                    return xq_sb[:, 2 * kp:2 * kp + 2, jc * P:(jc + 1) * P]
            else:
                kv_blocks = NCORES

                def k_mov(t, kp):
                    return xt_sb[:, t, 2 * kp:2 * kp + 2]

                def v_mov(t, kp, q):
                    return xt_sb[:, t, 2 * kp:2 * kp + 2, q * P:(q + 1) * P]

            with (
                tc.tile_pool(name="st2", bufs=2, space="PSUM") as st2_pool,
                tc.tile_pool(name="ctxp", bufs=1, space="PSUM") as ctx_pool,
                tc.tile_pool(name="csp", bufs=1, space="PSUM") as cs_pool,
                tc.tile_pool(name="scp", bufs=1, space="PSUM") as sc_pool,
                tc.tile_pool(name="etile", bufs=6) as etile,
                tc.tile_pool(name="mlp", bufs=2) as mlp,
            ):
                # ---- phase A ----
                # qT (+8 bq) FIRST: only needs xq/wq, runs while xt lands
                for it in range(NB // JT):
                    for dcp in range(DPAIR):
                        ps = st2_pool.tile([P, 2, JT], F32, tag="st")
                        for half in range(2):
                            dc = 2 * dcp + half
                            for kp in range(KP):
                                nc.tensor.matmul(
                                    ps[:, half],
                                    wq_sb[:, 2 * kp:2 * kp + 2,
                                          dc * P:(dc + 1) * P],
                                    xq_sb[:, 2 * kp:2 * kp + 2,
                                          it * JT:(it + 1) * JT],
                                    start=(kp == 0), stop=(kp == KP - 1),
                                    perf_mode=DR)
                        for half in range(2):
                            dc = 2 * dcp + half
                            nc.scalar.activation(
                                qt_sb[:, dc, it * JT:(it + 1) * JT],
                                ps[:, half], ACT.Identity,
                                bias=bq_sb[:, dc:dc + 1], scale=1.0)

                # kT / v'' per 512-column tile, paired with the xt DMA order
                for b in range(kv_blocks):
                    for jh in range(NB // JT):
                        t = 2 * b + jh
                        # kT (no bias; bk cancels in softmax)
                        for dcp in range(DPAIR):
                            ps = st2_pool.tile([P, 2, JT], F32, tag="st")
                            for half in range(2):
                                dc = 2 * dcp + half
                                for kp in range(KP):
                                    nc.tensor.matmul(
                                        ps[:, half],
                                        wk_sb[:, 2 * kp:2 * kp + 2,
                                              dc * P:(dc + 1) * P],
                                        k_mov(t, kp),
                                        start=(kp == 0), stop=(kp == KP - 1),
                                        perf_mode=DR)
                            nc.scalar.activation(
                                ktg[:, b, 2 * dcp:2 * dcp + 2,
                                    jh * JT:(jh + 1) * JT],
                                ps[:], ACT.Copy, bias=0.0, scale=1.0)
                        # v'' for the same 4 j-chunks
                        ps = st2_pool.tile([P, 4, DP2], F32, tag="st")
                        for q in range(4):
                            for kp in range(KP):
                                nc.tensor.matmul(
                                    ps[:, q],
                                    v_mov(t, kp, q),
                                    wv1_sb[:, 2 * kp:2 * kp + 2],
                                    start=(kp == 0), stop=(kp == KP - 1),
                                    perf_mode=DR)
                        nc.vector.tensor_tensor(
                            vg[:, b, 4 * jh:4 * jh + 4], ps[:],
                            bvp_sb[:], mybir.AluOpType.add)

                if exchange:
                    # ---- ready handshake + 14 XOR-addressed sends ----
                    nc.gpsimd.dma_start(margin_dram[:], ktg[0:1, 0, 0, 0:2])
                    nc.gpsimd.remote_sem_update_broadcast(
                        remote_sem=ready, local_sem=lsem,
                        rdests=[(0, k) for k in range(NCORES)])
                    nc.gpsimd.trigger_dma(count=1)
                    for m in range(1, NCORES):
                        rdests = [None] * 8
                        rdests[m] = (0, m)
                        nc.gpsimd.remote_dma_broadcast(
                            ktg[:, m], ktg[:, 0],
                            remote_sem=rsems[m - 1], local_sem=lsem,
                            rdests=rdests)
                        nc.gpsimd.remote_dma_broadcast(
                            vg[:, m], vg[:, 0],
                            remote_sem=rsems[m - 1], local_sem=lsem,
                            rdests=rdests)
                    w = nc.gpsimd.wait_ge(ready, 0)
                    patches.append((w, 2 * NCORES))
                    nc.gpsimd.trigger_dma(count=None)

                # ---- phase B: attention over the 8 blocks ----
                for ih in range(NIH):
                    i0 = ih * IH
                    ctx_ps = ctx_pool.tile([P, VC, IH], F32)
                    cs_ps = cs_pool.tile([32, IH], F32)
                    for s in range(NCORES):
                        if exchange and ih == 0 and s >= 1:
                            w = nc.tensor.wait_ge(rsems[s - 1], 0)
                            patches.append((w, 4))
                        for tp in range(BJP):
                            st = st2_pool.tile([P, 2, IH], F32, tag="st")
                            for half in range(2):
                                jc = 2 * tp + half
                                for dp in range(DPAIR):
                                    nc.tensor.matmul(
                                        st[:, half],
                                        ktg[:, s, 2 * dp:2 * dp + 2,
                                            jc * P:(jc + 1) * P],
                                        qt_sb[:, 2 * dp:2 * dp + 2,
                                              i0:i0 + IH],
                                        start=(dp == 0),
                                        stop=(dp == DPAIR - 1),
                                        perf_mode=DR)
                            e_t = etile.tile([P, 2, IH], FP8, tag="et")
                            nc.scalar.activation(e_t[:], st[:], ACT.Exp,
                                                 bias=0.0, scale=SCALE)
                            first = (s == 0 and tp == 0)
                            last = (s == NCORES - 1 and tp == BJP - 1)
                            nc.tensor.matmul(cs_ps[:], cs_w[:], e_t[:],
                                             start=first, stop=last,
                                             perf_mode=DR)
                            for vc in range(VC):
                                nc.tensor.matmul(
                                    ctx_ps[:, vc],
                                    vg[:, s, 2 * tp:2 * tp + 2,
                                       vc * P:(vc + 1) * P],
                                    e_t[:],
                                    start=first, stop=last,
                                    perf_mode=DR)

                    # tail: h = relu(ctxU''); raw = (W2/2)^T h; out = raw/den
                    h_sb = mlp.tile([P, VC, IH], BF16, tag="hsb")
                    sc_ps = sc_pool.tile([1, IH], F32, tag="sc")
                    for vc in range(VC):
                        nc.scalar.activation(h_sb[:, vc], ctx_ps[:, vc],
                                             ACT.Relu, bias=0.0, scale=1.0)
                        nc.tensor.matmul(sc_ps[:], w2_sb[:, vc:vc + 1],
                                         h_sb[:, vc],
                                         start=(vc == 0), stop=(vc == VC - 1))
                    recip = mlp.tile([1, IH], F32, tag="recip")
                    nc.vector.reciprocal_approx_fast(recip[:], cs_ps[0:1])
                    rawn = mlp.tile([1, IH], F32, tag="rawn")
                    nc.vector.tensor_tensor(rawn[:], sc_ps[:], recip[:],
                                            mybir.AluOpType.mult)
                    nc.scalar.add(out_sb[:, i0:i0 + IH], rawn[:], b2_sb[:])

            nc.sync.dma_start(out[:], out_sb[:])
            if exchange:
                # all sends flushed before teardown: 15 preps x 16
                w = nc.gpsimd.wait_ge(lsem, 0)
                patches.append((w, 15 * 16))

    if exchange:
        nc.has_collectives = True   # force NRT global comm init for RDMA
    for w, v in patches:
        w.ins.sync_info.on_wait[0].wait_value = v
    if my_sems:
        nc.clear_and_free_semaphores(my_sems)
    nc.finalize()
    return nc


def _prep(inputs):
    """Host-side layout prep: transposes, fp8 casts, Wv@W1 fold."""
    f32 = np.float32
    bf16 = ml_dtypes.bfloat16
    fp8 = ml_dtypes.float8_e4m3
    x = np.ascontiguousarray(inputs["x"], dtype=f32)
    xt_r = np.ascontiguousarray(
        x.T.reshape(KC, P, N).transpose(1, 0, 2).astype(fp8))    # [P, KC, N]

    def w_r(w, scale, cols):  # [IN, cols] -> [P, KC, cols], fp8 pre-scaled
        return np.ascontiguousarray(
            (np.asarray(w, f32) * scale).reshape(KC, P, cols)
            .transpose(1, 0, 2).astype(fp8))

    wv_w1 = np.asarray(inputs["Wv"], f32) @ np.asarray(inputs["W1"], f32)
    b1pp = (np.asarray(inputs["bv"], f32) @ np.asarray(inputs["W1"], f32)
            + np.asarray(inputs["b1"], f32))                     # [256]

    shared = {
        "wq": w_r(inputs["Wq"], WS, D),
        "wk": w_r(inputs["Wk"], WS, D),
        "wv1": w_r(wv_w1, WSV, DP2),
        "bq": np.ascontiguousarray(
            (np.asarray(inputs["bq"], f32) * WS).reshape(DC, P).T),
        "bvp": np.ascontiguousarray(
            np.broadcast_to(b1pp * WSV, (P, 4, DP2)).astype(f32)),
        "w2": np.ascontiguousarray(
            (np.asarray(inputs["W2"], f32) * 0.5)
            .reshape(VC, P).T.astype(bf16)),
        "b2": np.asarray(inputs["b2"], f32).reshape(1, 1),
    }
    if not EXCHANGE:
        # chunk-major [NT, P, KC, JT] so each 512-col tile DMAs contiguously
        NT = N // JT
        shared["xt"] = np.ascontiguousarray(
            xt_r.reshape(P, KC, NT, JT).transpose(2, 0, 1, 3))
    xqs = [np.ascontiguousarray(xt_r[:, :, c * NB:(c + 1) * NB])
           for c in range(NCORES)]
    return shared, xqs


def kernel(**inputs) -> np.ndarray:
    if "nc" not in _CACHE:
        _CACHE["nc"] = _build()
    nc = _CACHE["nc"]
    shared, xqs = _prep(inputs)
    in_maps = [dict(shared, xq=xqs[c]) for c in range(NCORES)]
    res = run_bass_kernel_spmd(nc, in_maps, core_ids=list(range(NCORES)))
    return np.concatenate([res.results[c]["out"][0] for c in range(NCORES)])


# revision 21
# speedup vs baseline: 1.5980x; 1.0052x over previous
"""CAAN attention-scorer kernel for 8 Trainium2 NeuronCores (v2).

scores = relu(softmax(QK^T/sqrt(D)) @ V @ W1 + b1) @ W2 + b2
with Q/K/V = x @ W{q,k,v} + b{q,k,v};  N=8192, IN_DIM=1024, D=512.

Math restructure vs v1 (all exact up to fp8 rounding):
  * Wv@W1 folded on host (attention rows sum to 1):
      V'' = x @ (Wv W1) + (bv W1 + b1)  [N, 256]
      h   = relu(attn @ V'')            -- the whole first MLP layer is gone
  * bk dropped: a per-row constant shift of the logits cancels in softmax.
  * softmax normalization deferred past relu (relu(a)/c == relu(a/c), c>0)
    to the final [1, N] score vector: scores = (W2/2)^T relu(ctxU'')*recip.

Sharding (EXCHANGE=True): row-parallel attention. Each core projects ONLY
its own 1024 rows of kT / V'' (1/8 of the work v1 replicated 8x) and the
shards are all-gathered SBUF-to-SBUF with 7 XOR-addressed
remote_dma_broadcast sends per core (slot s holds the block of core
own^s; attention is order-invariant over j so XOR order needs no
unpermute). Entry race (a peer's send landing before this core cleared
its semaphores) is closed by a ready-handshake gated on the first K
evacuation (~8us in, >> inter-core launch skew); data sends wait for
ready==16. Exchange semaphores are cleared at kernel end so repeated
executions see a clean state.

EXCHANGE=False fallback: every core computes the full kT / V'' from the
full x^T (v1-style replication) — no cross-core traffic.

Tile's scheduling simulator cannot model remotely-incremented semaphores,
so waits on them are emitted with threshold 0 and patched to the real
value after TileContext exits (before finalize).

Layouts (x8 / x16 are exact power-of-two fp8 pre-scales):
  kT[d, j]   = (8 Wk)^T x^T             d on partitions   (fp8)
  qT[d, i]   = (8 Wq)^T xq^T (+8 bq)                      (fp8)
  v''[j, d'] = x (16 WvW1) (+16 b1'')   j on partitions   (fp8)
  S^T[j, i]  = kT-chunk^T qT            (= 64 * true)
  E          = exp(S^T / (64 sqrt(D)))  PSUM -> SBUF fp8, [P, 1024] ops
  ctxU''    += v''-chunk^T E            (= 16 * true)
  den[1,i]  += (8.0)^T E                (= 8 * sum E)
  raw[1, i]  = (W2/2)^T relu(ctxU'')    (= 8 * true raw)
  out        = raw * (1/den) + b2
"""

import numpy as np
import ml_dtypes

import concourse.tile as tile
from concourse import bacc, mybir
from concourse.bass_utils import run_bass_kernel_spmd

N, IN_DIM, D = 8192, 1024, 512
DP2 = D // 2                # 256 folded v'' width
NCORES = 8
NB = N // NCORES            # 1024 rows per core
P = 128
KC = IN_DIM // P            # 8 k-chunks of the input dim
KP = KC // 2                # 4 DoubleRow k-pairs
DC = D // P                 # 4 d-chunks (q/k)
DPAIR = DC // 2             # 2 DoubleRow d-pairs
VC = DP2 // P               # 2 d'-chunks (v'')
JT = 512                    # j-tile width in phase A
BJC = NB // P               # 8 j-chunks per block
BJP = BJC // 2              # 4 j-pairs per block
IH = 512                    # i-half width in phase B
NIH = NB // IH              # 2
WS = 8.0                    # fp8 weight pre-scale for Wq/Wk
WSV = 16.0                  # fp8 pre-scale for folded WvW1
SCALE = 1.0 / float(np.sqrt(np.float32(D))) / (WS * WS)

FP8 = mybir.dt.float8e4
BF16 = mybir.dt.bfloat16
F32 = mybir.dt.float32
DR = mybir.MatmulPerfMode.DoubleRow
ACT = mybir.ActivationFunctionType

EXCHANGE = False

_CACHE = {}


def _build(exchange=EXCHANGE):
    nc = bacc.Bacc(None, target_bir_lowering=False, debug=False,
                   num_devices=NCORES, use_seq_codegen=True)

    NT = N // JT
    xq = nc.declare_dram_parameter("xq", [P, KC, NB], FP8, isOutput=False)
    if not exchange:
        # chunk-major so each 512-column tile is one contiguous DMA
        xt = nc.declare_dram_parameter("xt", [NT, P, KC, JT], FP8,
                                       isOutput=False)
    wq = nc.declare_dram_parameter("wq", [P, KC, D], FP8, isOutput=False)
    wk = nc.declare_dram_parameter("wk", [P, KC, D], FP8, isOutput=False)
    wv1 = nc.declare_dram_parameter("wv1", [P, KC, DP2], FP8, isOutput=False)
    bq = nc.declare_dram_parameter("bq", [P, DC], F32, isOutput=False)   # x8
    bvp = nc.declare_dram_parameter("bvp", [P, 4, DP2], F32,
                                    isOutput=False)  # x16
    w2 = nc.declare_dram_parameter("w2", [P, VC], BF16, isOutput=False)  # x0.5
    b2 = nc.declare_dram_parameter("b2", [1, 1], F32, isOutput=False)
    out = nc.declare_dram_parameter("out", [1, NB], F32, isOutput=True)

    patches = []   # (BassInstruction, real_wait_value)
    my_sems = []
    with tile.TileContext(nc) as tc:
        with tc.tile_pool(name="singles", bufs=1) as singles:
            # ---- weights / constants into SBUF ----
            wk_sb = singles.tile([P, KC, D], FP8)
            wq_sb = singles.tile([P, KC, D], FP8)
            wv1_sb = singles.tile([P, KC, DP2], FP8)
            bq_sb = singles.tile([P, DC], F32)
            bvp_sb = singles.tile([P, 4, DP2], F32)
            w2_sb = singles.tile([P, VC], BF16)
            b2_sb = singles.tile([1, 1], F32)
            cs_w = singles.tile([P, 2, 32], FP8)   # colsum weights = 8.0
            xq_sb = singles.tile([P, KC, NB], FP8)
            if not exchange:
                xt_sb = singles.tile([P, NT, KC, JT], FP8)
            qt_sb = singles.tile([P, DC, NB], FP8)
            # gathered kT / v'': slot s = block of core (own XOR s)
            # (global block s when exchange=False)
            ktg = singles.tile([P, NCORES, DC, NB], FP8)
            vg = singles.tile([P, NCORES, BJC, DP2], FP8)
            out_sb = singles.tile([1, NB], F32)

            # DMA order tuned so Q can start immediately and kt follows:
            # xq (sync) + wq/bq (gpsimd) land first, wk next, then xt.
            for kp in range(KP):
                nc.sync.dma_start(xq_sb[:, 2 * kp:2 * kp + 2],
                                  xq[:, 2 * kp:2 * kp + 2])
            for kp in range(KP):
                nc.gpsimd.dma_start(out=wq_sb[:, 2 * kp:2 * kp + 2],
                                    in_=wq[:, 2 * kp:2 * kp + 2])
            nc.gpsimd.dma_start(out=bq_sb[:], in_=bq[:])
            for kp in range(KP):
                nc.gpsimd.dma_start(out=wk_sb[:, 2 * kp:2 * kp + 2],
                                    in_=wk[:, 2 * kp:2 * kp + 2])
            nc.gpsimd.dma_start(out=wv1_sb[:], in_=wv1[:])
            nc.gpsimd.dma_start(out=bvp_sb[:], in_=bvp[:])
            nc.gpsimd.dma_start(out=w2_sb[:], in_=w2[:])
            nc.gpsimd.dma_start(out=b2_sb[:], in_=b2[:])
            nc.vector.memset(cs_w[:], WS)
            if not exchange:
                for t in range(NT):
                    nc.sync.dma_start(xt_sb[:, t], xt[t])

            if exchange:
                rsems = [nc.alloc_semaphore(f"xch_{m}")
                         for m in range(1, NCORES)]
                ready = nc.alloc_semaphore("xch_ready")
                lsem = nc.alloc_semaphore("xch_local")
                my_sems = rsems + [ready, lsem]
                margin_dram = nc.dram_tensor("margin_scratch", [1, 2], FP8)

            # j-source slices for the kT / v'' projections
            if exchange:
                kv_blocks = 1

                def k_mov(t, kp):
                    return xq_sb[:, 2 * kp:2 * kp + 2,
                                 (t % 2) * JT:(t % 2 + 1) * JT]

                def v_mov(t, kp, q):
                    jc = (t % 2) * 4 + q
                    return xq_sb[:, 2 * kp:2 * kp + 2, jc * P:(jc + 1) * P]
            else:
                kv_blocks = NCORES

                def k_mov(t, kp):
                    return xt_sb[:, t, 2 * kp:2 * kp + 2]

                def v_mov(t, kp, q):
                    return xt_sb[:, t, 2 * kp:2 * kp + 2, q * P:(q + 1) * P]

            with (
                tc.tile_pool(name="st2", bufs=2, space="PSUM") as st2_pool,
                tc.tile_pool(name="ctxp", bufs=1, space="PSUM") as ctx_pool,
                tc.tile_pool(name="csp", bufs=1, space="PSUM") as cs_pool,
                tc.tile_pool(name="scp", bufs=1, space="PSUM") as sc_pool,
                tc.tile_pool(name="etile", bufs=6) as etile,
                tc.tile_pool(name="mlp", bufs=2) as mlp,
            ):
                # ---- phase A ----
                # qT (+8 bq) FIRST: only needs xq/wq, runs while xt lands
                for it in range(NB // JT):
                    for dcp in range(DPAIR):
                        ps = st2_pool.tile([P, 2, JT], F32, tag="st")
                        for half in range(2):
                            dc = 2 * dcp + half
                            for kp in range(KP):
                                nc.tensor.matmul(
                                    ps[:, half],
                                    wq_sb[:, 2 * kp:2 * kp + 2,
                                          dc * P:(dc + 1) * P],
                                    xq_sb[:, 2 * kp:2 * kp + 2,
                                          it * JT:(it + 1) * JT],
                                    start=(kp == 0), stop=(kp == KP - 1),
                                    perf_mode=DR)
                        for half in range(2):
                            dc = 2 * dcp + half
                            nc.scalar.activation(
                                qt_sb[:, dc, it * JT:(it + 1) * JT],
                                ps[:, half], ACT.Identity,
                                bias=bq_sb[:, dc:dc + 1], scale=1.0)

                # kT / v'' per 512-column tile, paired with the xt DMA order
                for b in range(kv_blocks):
                    for jh in range(NB // JT):
                        t = 2 * b + jh
                        # kT (no bias; bk cancels in softmax)
                        for dcp in range(DPAIR):
                            ps = st2_pool.tile([P, 2, JT], F32, tag="st")
                            for half in range(2):
                                dc = 2 * dcp + half
                                for kp in range(KP):
                                    nc.tensor.matmul(
                                        ps[:, half],
                                        wk_sb[:, 2 * kp:2 * kp + 2,
                                              dc * P:(dc + 1) * P],
                                        k_mov(t, kp),
                                        start=(kp == 0), stop=(kp == KP - 1),
                                        perf_mode=DR)
                            nc.scalar.activation(
                                ktg[:, b, 2 * dcp:2 * dcp + 2,
                                    jh * JT:(jh + 1) * JT],
                                ps[:], ACT.Copy, bias=0.0, scale=1.0)
                        # v'' for the same 4 j-chunks
                        ps = st2_pool.tile([P, 4, DP2], F32, tag="st")
                        for q in range(4):
                            for kp in range(KP):
                                nc.tensor.matmul(
                                    ps[:, q],
                                    v_mov(t, kp, q),
                                    wv1_sb[:, 2 * kp:2 * kp + 2],
                                    start=(kp == 0), stop=(kp == KP - 1),
                                    perf_mode=DR)
                        nc.vector.tensor_tensor(
                            vg[:, b, 4 * jh:4 * jh + 4], ps[:],
                            bvp_sb[:], mybir.AluOpType.add)

                if exchange:
                    # ---- ready handshake + 14 XOR-addressed sends ----
                    nc.gpsimd.dma_start(margin_dram[:], ktg[0:1, 0, 0, 0:2])
                    nc.gpsimd.remote_sem_update_broadcast(
                        remote_sem=ready, local_sem=lsem,
                        rdests=[(0, k) for k in range(NCORES)])
                    nc.gpsimd.trigger_dma(count=1)
                    for m in range(1, NCORES):
                        rdests = [None] * 8
                        rdests[m] = (0, m)
                        nc.gpsimd.remote_dma_broadcast(
                            ktg[:, m], ktg[:, 0],
                            remote_sem=rsems[m - 1], local_sem=lsem,
                            rdests=rdests)
                        nc.gpsimd.remote_dma_broadcast(
                            vg[:, m], vg[:, 0],
                            remote_sem=rsems[m - 1], local_sem=lsem,
                            rdests=rdests)
                    w = nc.gpsimd.wait_ge(ready, 0)
                    patches.append((w, 2 * NCORES))
                    nc.gpsimd.trigger_dma(count=None)

                # ---- phase B: attention over the 8 blocks ----
                for ih in range(NIH):
                    i0 = ih * IH
                    ctx_ps = ctx_pool.tile([P, VC, IH], F32)
                    cs_ps = cs_pool.tile([32, IH], F32)
                    for s in range(NCORES):
                        if exchange and ih == 0 and s >= 1:
                            w = nc.tensor.wait_ge(rsems[s - 1], 0)
                            patches.append((w, 4))
                        for tp in range(BJP):
                            st = st2_pool.tile([P, 2, IH], F32, tag="st")
                            for half in range(2):
                                jc = 2 * tp + half
                                for dp in range(DPAIR):
                                    nc.tensor.matmul(
                                        st[:, half],
                                        ktg[:, s, 2 * dp:2 * dp + 2,
                                            jc * P:(jc + 1) * P],
                                        qt_sb[:, 2 * dp:2 * dp + 2,
                                              i0:i0 + IH],
                                        start=(dp == 0),
                                        stop=(dp == DPAIR - 1),
                                        perf_mode=DR)
                            e_t = etile.tile([P, 2, IH], FP8, tag="et")
                            nc.scalar.activation(e_t[:], st[:], ACT.Exp,
                                                 bias=0.0, scale=SCALE)
                            first = (s == 0 and tp == 0)
                            last = (s == NCORES - 1 and tp == BJP - 1)
                            nc.tensor.matmul(cs_ps[:], cs_w[:], e_t[:],
                                             start=first, stop=last,
                                             perf_mode=DR)
                            for vc in range(VC):
                                nc.tensor.matmul(
                                    ctx_ps[:, vc],
                                    vg[:, s, 2 * tp:2 * tp + 2,
                                       vc * P:(vc + 1) * P],
                                    e_t[:],
                                    start=first, stop=last,
                                    perf_mode=DR)

                    # tail: h = relu(ctxU''); raw = (W2/2)^T h; out = raw/den
                    h_sb = mlp.tile([P, VC, IH], BF16, tag="hsb")
                    sc_ps = sc_pool.tile([1, IH], F32, tag="sc")
                    for vc in range(VC):
                        nc.scalar.activation(h_sb[:, vc], ctx_ps[:, vc],
                                             ACT.Relu, bias=0.0, scale=1.0)
                        nc.tensor.matmul(sc_ps[:], w2_sb[:, vc:vc + 1],
                                         h_sb[:, vc],
                                         start=(vc == 0), stop=(vc == VC - 1))
                    recip = mlp.tile([1, IH], F32, tag="recip")
                    nc.vector.reciprocal_approx_fast(recip[:], cs_ps[0:1])
                    rawn = mlp.tile([1, IH], F32, tag="rawn")
                    nc.vector.tensor_tensor(rawn[:], sc_ps[:], recip[:],
                                            mybir.AluOpType.mult)
                    nc.scalar.add(out_sb[:, i0:i0 + IH], rawn[:], b2_sb[:])

            nc.sync.dma_start(out[:], out_sb[:])
            if exchange:
                # all sends flushed before teardown: 15 preps x 16
                w = nc.gpsimd.wait_ge(lsem, 0)
                patches.append((w, 15 * 16))

    if exchange:
        nc.has_collectives = True   # force NRT global comm init for RDMA
    for w, v in patches:
        w.ins.sync_info.on_wait[0].wait_value = v
    if my_sems:
        nc.clear_and_free_semaphores(my_sems)
    nc.finalize()
    return nc


def _prep(inputs):
    """Host-side layout prep: transposes, fp8 casts, Wv@W1 fold."""
    f32 = np.float32
    bf16 = ml_dtypes.bfloat16
    fp8 = ml_dtypes.float8_e4m3
    x = np.ascontiguousarray(inputs["x"], dtype=f32)
    xt_r = np.ascontiguousarray(
        x.T.reshape(KC, P, N).transpose(1, 0, 2).astype(fp8))    # [P, KC, N]

    def w_r(w, scale, cols):  # [IN, cols] -> [P, KC, cols], fp8 pre-scaled
        return np.ascontiguousarray(
            (np.asarray(w, f32) * scale).reshape(KC, P, cols)
            .transpose(1, 0, 2).astype(fp8))

    wv_w1 = np.asarray(inputs["Wv"], f32) @ np.asarray(inputs["W1"], f32)
    b1pp = (np.asarray(inputs["bv"], f32) @ np.asarray(inputs["W1"], f32)
            + np.asarray(inputs["b1"], f32))                     # [256]

    shared = {
        "wq": w_r(inputs["Wq"], WS, D),
        "wk": w_r(inputs["Wk"], WS, D),
        "wv1": w_r(wv_w1, WSV, DP2),
        "bq": np.ascontiguousarray(
            (np.asarray(inputs["bq"], f32) * WS).reshape(DC, P).T),
        "bvp": np.ascontiguousarray(
            np.broadcast_to(b1pp * WSV, (P, 4, DP2)).astype(f32)),
        "w2": np.ascontiguousarray(
            (np.asarray(inputs["W2"], f32) * 0.5)
            .reshape(VC, P).T.astype(bf16)),
        "b2": np.asarray(inputs["b2"], f32).reshape(1, 1),
    }
    if not EXCHANGE:
        # chunk-major [NT, P, KC, JT] so each 512-col tile DMAs contiguously
        NT = N // JT
        shared["xt"] = np.ascontiguousarray(
            xt_r.reshape(P, KC, NT, JT).transpose(2, 0, 1, 3))
    xqs = [np.ascontiguousarray(xt_r[:, :, c * NB:(c + 1) * NB])
           for c in range(NCORES)]
    return shared, xqs


def kernel(**inputs) -> np.ndarray:
    if "nc" not in _CACHE:
        _CACHE["nc"] = _build()
    nc = _CACHE["nc"]
    shared, xqs = _prep(inputs)
    in_maps = [dict(shared, xq=xqs[c]) for c in range(NCORES)]
    res = run_bass_kernel_spmd(nc, in_maps, core_ids=list(range(NCORES)))
    return np.concatenate([res.results[c]["out"][0] for c in range(NCORES)])
